# revision 14
# baseline (speedup 1.0000x reference)
"""Trainium2 Bass kernel for nn_CRSDCell_84774064488468.

Self-contained: hardcodes shapes (B=2,T=2048,D=512,N=16,DK=DV=64) and the
8-core sharding. Strategy:
  - P0: LayerNorm is data-parallel over 8 bt-slices of 512; normalized x
    (gamma/beta folded into downstream weights) is transposed on-chip and
    AllGathered so every core holds xn^T [512d, 4096bt] in fp32.
  - The selective-scan state dim (d,n) is sharded: core c owns dn slice
    [1024c, 1024(c+1)) (d-major), so the T-scan needs no cross-core carries
    and the B_cont mean over (B,T) needs no collective.
  - The reference's exclusive prefix-product makes its recurrence
    h_t = h_{t-1}/A_bar_{t-1} + B_bar_t; A_log is constant so the decay is
    rank-1: scan coefficients are rows exp(+/-exp(A_log)*delta_t) broadcast
    across partitions. The scan itself is one tensor_tensor_scan per
    (dn-block, chunk) on the vector engine.
  - B matmul runs twice (once for the mean, once fused into the scan stream)
    to keep the h path fully fp32 without 16MiB of B_cont residency.
  - h_seq and y are produced transposed ([dn, bt] / [d, bt]) so all HBM
    writes are contiguous; the host un-transposes while unsharding.
  - KCM tail: y_ssm^T is AllToAll'd so each core gets full d' for its
    bt-slice and finishes k/v_hat/h_mem/gate locally.
"""
import numpy as np

B, T, DX, DH, N, DK, DV = 2, 2048, 512, 512, 16, 64, 64
BT = B * T
W = 8
SL = BT // W          # 512 bt per core (LN/KCM slice)
DSH = DH // W         # 64 d' per core
DNSH = DSH * N        # 1024 dn per core
NJ = DNSH // 128      # 8 dn blocks
NQ = BT // 512        # 8 bt chunks
RG = [[0, 1, 2, 3, 4, 5, 6, 7]]

_CACHE = {}


def _build(a0, db_f):
    """Build the SPMD Bass program. a0 = A_cont scalar (negative), db_f =
    folded delta bias. Returns nc."""
    import concourse.mybir as mybir
    from concourse import tile, bacc
    from contextlib import ExitStack

    dt = mybir.dt
    f32, bf16 = dt.float32, dt.bfloat16
    AF = mybir.ActivationFunctionType
    ALU = mybir.AluOpType
    r0 = 0.5 / a0

    nc = bacc.Bacc()

    # ---------------- DRAM tensors ----------------
    x_sl = nc.dram_tensor("x_sl", [SL, DX], f32, kind="ExternalInput")
    bw = nc.dram_tensor("bw", [DX, DNSH], f32, kind="ExternalInput")
    cw = nc.dram_tensor("cw", [DX, DNSH], f32, kind="ExternalInput")
    dwp = nc.dram_tensor("dwp", [DX, DSH], f32, kind="ExternalInput")
    dwd = nc.dram_tensor("dwd", [DX, 1], f32, kind="ExternalInput")
    bbf = nc.dram_tensor("bbf", [128, NJ], f32, kind="ExternalInput")
    cbf = nc.dram_tensor("cbf", [128, NJ], f32, kind="ExternalInput")
    dbcol = nc.dram_tensor("dbcol", [DSH, 1], f32, kind="ExternalInput")
    dccol = nc.dram_tensor("dccol", [DSH, 1], f32, kind="ExternalInput")
    kgw = nc.dram_tensor("kgw", [DH, DK + 1], f32, kind="ExternalInput")
    kbcol = nc.dram_tensor("kbcol", [DK + 1, 1], f32, kind="ExternalInput")
    mw = nc.dram_tensor("mw", [DK, DV], f32, kind="ExternalInput")
    rcw = nc.dram_tensor("rcw", [DV, DH], f32, kind="ExternalInput")
    rbcol = nc.dram_tensor("rbcol", [128, 4], f32, kind="ExternalInput")
    smat = nc.dram_tensor("smat", [128, 512], f32, kind="ExternalInput")
    identf = nc.dram_tensor("identf", [128, 128], f32, kind="ExternalInput")

    h_T = nc.dram_tensor("h_T", [DNSH, BT], f32, kind="ExternalOutput")
    y_T = nc.dram_tensor("y_T", [DH, SL], f32, kind="ExternalOutput")

    cc_in = nc.dram_tensor("cc_in", [4 * 128 + 3, SL], f32, kind="Internal")
    cc_out = nc.dram_tensor("cc_out", [W, 4 * 128 + 3, SL], f32, kind="Internal",
                            addr_space="Shared")
    cc2_in = nc.dram_tensor("cc2_in", [W, DSH, SL], f32, kind="Internal")
    cc2_out = nc.dram_tensor("cc2_out", [W, DSH, SL], f32, kind="Internal")

    with tile.TileContext(nc) as tc, ExitStack() as ctx:
        pc = ctx.enter_context(tc.tile_pool(name="consts", bufs=1))
        pmain = ctx.enter_context(tc.tile_pool(name="main", bufs=1))

        # ---- constants to SBUF ----
        identf_t = pc.tile([128, 128], f32)
        nc.sync.dma_start(identf_t[:], identf[:])
        bbf_t = pc.tile([128, NJ], f32)
        nc.sync.dma_start(bbf_t[:], bbf[:])
        cbf_t = pc.tile([128, NJ], f32)
        nc.sync.dma_start(cbf_t[:], cbf[:])
        dbcol_t = pc.tile([DSH, 1], f32)
        nc.sync.dma_start(dbcol_t[:], dbcol[:])
        dccol_t = pc.tile([DSH, 1], f32)
        nc.sync.dma_start(dccol_t[:], dccol[:])
        smat_t = pc.tile([128, 512], f32)
        nc.sync.dma_start(smat_t[:], smat[:])
        kg_t = [pc.tile([128, DK + 1], f32, name=f"kg{k}", tag=f"kg{k}") for k in range(4)]
        for k in range(4):
            nc.sync.dma_start(kg_t[k][:], kgw[k * 128:(k + 1) * 128, :])
        kbcol_t = pc.tile([DK + 1, 1], f32)
        nc.sync.dma_start(kbcol_t[:], kbcol[:])
        mw_t = pc.tile([DK, DV], f32)
        nc.sync.dma_start(mw_t[:], mw[:])
        rcw_t = [pc.tile([DV, 128], f32, name=f"rc{m}", tag=f"rc{m}") for m in range(4)]
        for m in range(4):
            nc.sync.dma_start(rcw_t[m][:], rcw[:, m * 128:(m + 1) * 128])
        rbcol_t = pc.tile([128, 4], f32)
        nc.sync.dma_start(rbcol_t[:], rbcol[:])
        dwd_t = [pc.tile([128, 1], f32, name=f"dwd{k}", tag=f"dwd{k}") for k in range(4)]
        for k in range(4):
            nc.sync.dma_start(dwd_t[k][:], dwd[k * 128:(k + 1) * 128, :])
        dwp_t = [pc.tile([128, DSH], f32, name=f"dwp{k}", tag=f"dwp{k}") for k in range(4)]
        for k in range(4):
            nc.sync.dma_start(dwp_t[k][:], dwp[k * 128:(k + 1) * 128, :])

        # ---- persistent big tiles ----
        xn = [pmain.tile([128, BT], f32, name=f"xn{k}", tag=f"xn{k}") for k in range(4)]
        # engine APs may only start at partitions 0/32/64/96:
        # rowsA p0=delta, p32=ainv, p64=am1r, p96=mu; rowsB p0=istd
        rowsA = pmain.tile([97, BT], f32, tag="rowsA")
        rowsB = pmain.tile([1, BT], f32, tag="rowsB")
        ydss = pmain.tile([128, BT], f32, tag="ydss")  # 0:64 yD, 64:128 ySSM
        carry = pmain.tile([128, NJ], f32, tag="carry")
        Bacc = pmain.tile([128, NJ * NQ], f32, tag="bacc")
        nmc = pmain.tile([128, NJ], f32, tag="nmc")    # -mean per dn col

        # ================= P0: LN on slice + transpose + delta =================
        with tc.tile_pool(name="p0", bufs=2) as p0, \
             tc.tile_pool(name="p0s", bufs=8) as p0s, \
             tc.tile_pool(name="p0c", bufs=1) as p0c, \
             tc.tile_pool(name="p0ps", bufs=3, space="PSUM") as p0ps:
            xnsl = [p0c.tile([128, SL], f32, name=f"xnsl{s}", tag=f"xnsl{s}") for s in range(4)]
            eps_col = p0c.tile([128, 1], f32, tag="eps_col")
            nc.vector.memset(eps_col[:], 1e-5)
            dbf_col = p0c.tile([1, 1], f32, tag="dbf_col")
            nc.vector.memset(dbf_col[:], db_f)
            colcat = p0c.tile([128, 8], f32, tag="colcat")
            stat8 = p0c.tile([8, 128], f32, tag="stat8")
            dmin = p0c.tile([1, SL], f32, tag="dmin")
            for ti in range(4):
                xt = p0.tile([128, DX], f32, tag="xt")
                nc.sync.dma_start(xt[:], x_sl[ti * 128:(ti + 1) * 128, :])
                scr = p0.tile([128, DX], f32, tag="scr")
                asum = p0s.tile([128, 1], f32, tag="asum")
                nc.scalar.activation(scr[:], xt[:], AF.Copy, accum_out=asum[:])
                negmean = p0s.tile([128, 1], f32, tag="negmean")
                nc.vector.tensor_scalar_mul(negmean[:], asum[:], -1.0 / DX)
                ssq = p0s.tile([128, 1], f32, tag="ssq")
                nc.scalar.activation(scr[:], xt[:], AF.Square, bias=negmean[:],
                                     accum_out=ssq[:])
                istd = p0s.tile([128, 1], f32, tag="istd")
                nc.scalar.activation(istd[:], ssq[:], AF.Sqrt, scale=1.0 / DX,
                                     bias=eps_col[:])
                rstd = p0s.tile([128, 1], f32, tag="rstd")
                nc.vector.reciprocal(rstd[:], istd[:])
                xnb = p0.tile([128, DX], f32, tag="xnb")
                nc.vector.tensor_scalar(xnb[:], xt[:], negmean[:], rstd[:],
                                        ALU.add, ALU.mult)
                nc.vector.tensor_scalar_mul(colcat[:, ti:ti + 1], negmean[:], -1.0)
                nc.vector.tensor_copy(colcat[:, 4 + ti:5 + ti], istd[:])
                for s in range(4):
                    ptr = p0ps.tile([128, 128], f32, tag="ptr")
                    nc.tensor.transpose(ptr[:], xnb[:, s * 128:(s + 1) * 128],
                                        identf_t[:])
                    nc.scalar.activation(xnsl[s][:, ti * 128:(ti + 1) * 128],
                                         ptr[:], AF.Copy)
            # delta on the slice
            pd = p0ps.tile([1, SL], f32, tag="pd", bufs=1)
            for k in range(4):
                nc.tensor.matmul(pd[:], dwd_t[k][:], xnsl[k][:],
                                 start=(k == 0), stop=(k == 3))
            # softplus = ln(1+exp(.)) — Softplus has no ACT table in this build
            sp = p0s.tile([1, SL], f32, tag="sp")
            nc.scalar.activation(sp[:], pd[:], AF.Exp, bias=dbf_col[:])
            nc.vector.tensor_scalar(sp[:], sp[:], 1.0, None, ALU.add)
            nc.scalar.activation(sp[:], sp[:], AF.Ln)
            nc.vector.tensor_scalar(dmin[:], sp[:], 0.5, None, ALU.min)
            # mu/istd cols -> rows via PE transpose + whole-tile copy + DMA
            pcol = p0ps.tile([8, 128], f32, tag="pcol", bufs=1)
            nc.tensor.transpose(pcol[:], colcat[:], identf_t[:])
            nc.scalar.activation(stat8[:], pcol[:], AF.Copy)
            for s in range(4):
                nc.sync.dma_start(cc_in[s * 128:(s + 1) * 128, :], xnsl[s][:])
            nc.sync.dma_start(cc_in[512:513, :], dmin[:])
            nc.sync.dma_start(cc_in[513, :].rearrange("(a b) -> a b", a=4),
                              stat8[0:4, :])
            nc.sync.dma_start(cc_in[514, :].rearrange("(a b) -> a b", a=4),
                              stat8[4:8, :])

        # ================= AllGather =================
        nc.gpsimd.collective_compute(
            "AllGather", ALU.bypass,
            ins=[cc_in[:]],
            outs=[cc_out[:]],
            replica_groups=RG,
        )
        for k in range(4):
            for c in range(W):
                nc.sync.dma_start(xn[k][:, c * SL:(c + 1) * SL],
                                  cc_out[c, k * 128:(k + 1) * 128, :])
        for c in range(W):
            nc.sync.dma_start(rowsA[0:1, c * SL:(c + 1) * SL],
                              cc_out[c, 512:513, :])
            nc.sync.dma_start(rowsA[96:97, c * SL:(c + 1) * SL],
                              cc_out[c, 513:514, :])
            nc.sync.dma_start(rowsB[0:1, c * SL:(c + 1) * SL],
                              cc_out[c, 514:515, :])

        # decay rows (rank-1: A_cont is a constant scalar a0)
        # ainv (shifted): rowsA[32, t] = exp(-a0*delta[t-1]), 1.0 at b starts
        nc.scalar.activation(rowsA[32:33, 1:BT], rowsA[0:1, 0:BT - 1], AF.Exp,
                             scale=-a0)
        # am1r: rowsA[64, t] = (exp(a0*delta[t]) - 1) * r0
        nc.scalar.activation(rowsA[64:65, :], rowsA[0:1, :], AF.Exp, scale=a0)
        nc.vector.memset(rowsA[32:33, 0:1], 1.0)
        nc.vector.memset(rowsA[32:33, T:T + 1], 1.0)
        nc.vector.tensor_scalar(rowsA[64:65, :], rowsA[64:65, :], 1.0, r0,
                                ALU.subtract, ALU.mult)

        # ================= P1: B matmul #1 (mean) + D proj =================
        with tc.tile_pool(name="wb", bufs=8) as pwb, \
             tc.tile_pool(name="psB", bufs=3, space="PSUM") as ppsB, \
             tc.tile_pool(name="psD", bufs=2, space="PSUM") as ppsD, \
             tc.tile_pool(name="bcd", bufs=4) as pbcd, \
             tc.tile_pool(name="sc1", bufs=2) as psc1:
            for j in range(NJ):
                wts = []
                for k in range(4):
                    wt = pwb.tile([128, 128], f32, tag="wb")
                    nc.sync.dma_start(
                        wt[:], bw[k * 128:(k + 1) * 128, j * 128:(j + 1) * 128])
                    wts.append(wt)
                for q in range(NQ):
                    ps = ppsB.tile([128, 512], f32, tag="psB")
                    for k in range(4):
                        nc.tensor.matmul(ps[:], wts[k][:],
                                         xn[k][:, q * 512:(q + 1) * 512],
                                         start=(k == 0), stop=(k == 3))
                    sco = psc1.tile([128, 512], f32, tag="sco")
                    nc.scalar.activation(
                        sco[:], ps[:], AF.Silu, bias=bbf_t[:, j:j + 1],
                        accum_out=Bacc[:, j * NQ + q:j * NQ + q + 1])
            # D projection (d' slice), with LN inversion corrections
            for q in range(NQ):
                qsl = slice(q * 512, (q + 1) * 512)
                psd = ppsD.tile([DSH, 512], f32, tag="psD")
                for k in range(4):
                    nc.tensor.matmul(psd[:], dwp_t[k][:], xn[k][:, qsl],
                                     start=(k == 0), stop=(k == 3))
                mu_s = pbcd.tile([1, 512], f32, tag="mu_s")
                nc.scalar.activation(mu_s[:], rowsA[96:97, qsl], AF.Copy)
                mub = pbcd.tile([128, 512], f32, tag="mub")
                nc.gpsimd.partition_broadcast(mub[:], mu_s[:])
                isb = pbcd.tile([128, 512], f32, tag="isb")
                nc.gpsimd.partition_broadcast(isb[:], rowsB[0:1, qsl])
                t1 = psc1.tile([DSH, 512], f32, tag="t1")
                nc.vector.scalar_tensor_tensor(t1[:], psd[:], 1.0, isb[0:DSH, :],
                                               ALU.mult, ALU.mult)
                nc.vector.scalar_tensor_tensor(ydss[0:DSH, qsl], mub[0:DSH, :],
                                               dccol_t[:], t1[:],
                                               ALU.mult, ALU.add)
            # -mean cols per dn block
            for j in range(NJ):
                nc.vector.tensor_reduce(nmc[:, j:j + 1],
                                        Bacc[:, j * NQ:(j + 1) * NQ],
                                        mybir.AxisListType.X, ALU.add)
                nc.vector.tensor_scalar_mul(nmc[:, j:j + 1], nmc[:, j:j + 1],
                                            -1.0 / BT)

        # ================= P2: scan stream + B2/C matmuls =================
        with tc.tile_pool(name="wc", bufs=32) as pwc, \
             tc.tile_pool(name="wb2", bufs=32) as pwb2, \
             tc.tile_pool(name="psB2", bufs=2, space="PSUM") as ppsB2, \
             tc.tile_pool(name="psC", bufs=2, space="PSUM") as ppsC, \
             tc.tile_pool(name="psH", bufs=2, space="PSUM") as ppsH, \
             tc.tile_pool(name="st", bufs=3) as pst, \
             tc.tile_pool(name="sth", bufs=3) as psth, \
             tc.tile_pool(name="bc2", bufs=3) as pbc2:
            wcts, wb2 = {}, {}
            for j in range(NJ):
                for k in range(4):
                    wt = pwc.tile([128, 128], f32, tag="wc")
                    nc.sync.dma_start(
                        wt[:], cw[k * 128:(k + 1) * 128, j * 128:(j + 1) * 128])
                    wcts[(j, k)] = wt
                    wt = pwb2.tile([128, 128], f32, tag="wb2")
                    nc.sync.dma_start(
                        wt[:], bw[k * 128:(k + 1) * 128, j * 128:(j + 1) * 128])
                    wb2[(j, k)] = wt
            for q in range(NQ):
                qsl = slice(q * 512, (q + 1) * 512)
                ai_s = pbc2.tile([1, 512], f32, tag="ai_s")
                nc.scalar.activation(ai_s[:], rowsA[32:33, qsl], AF.Copy)
                abc = pbc2.tile([128, 512], f32, tag="abc")
                nc.gpsimd.partition_broadcast(abc[:], ai_s[:])
                am_s = pbc2.tile([1, 512], f32, tag="am_s")
                nc.scalar.activation(am_s[:], rowsA[64:65, qsl], AF.Copy)
                amb = pbc2.tile([128, 512], f32, tag="amb")
                nc.gpsimd.partition_broadcast(amb[:], am_s[:])
                psh = ppsH.tile([DSH, 512], f32, tag="psH")
                for j in range(NJ):
                    # --- B matmul #2 -> fp32 B_cont transient ---
                    psb = ppsB2.tile([128, 512], f32, tag="psB2")
                    for k in range(4):
                        nc.tensor.matmul(psb[:], wb2[(j, k)][:], xn[k][:, qsl],
                                         start=(k == 0), stop=(k == 3))
                    bco = pst.tile([128, 512], f32, tag="bco")
                    nc.scalar.activation(bco[:], psb[:], AF.Silu,
                                         bias=bbf_t[:, j:j + 1])
                    # --- C matmul + silu (fp32 transient) ---
                    psc = ppsC.tile([128, 512], f32, tag="psC")
                    for k in range(4):
                        nc.tensor.matmul(psc[:], wcts[(j, k)][:], xn[k][:, qsl],
                                         start=(k == 0), stop=(k == 3))
                    cct = pst.tile([128, 512], f32, tag="cct")
                    nc.scalar.activation(cct[:], psc[:], AF.Silu,
                                         bias=cbf_t[:, j:j + 1])
                    # --- bbar = (B_cont - mean) * am1 * r0 ---
                    bb = pst.tile([128, 512], f32, tag="bb")
                    nc.vector.scalar_tensor_tensor(bb[:], bco[:], nmc[:, j:j + 1],
                                                   amb[:], ALU.add, ALU.mult)
                    # --- scan ---
                    ht = psth.tile([128, 512], f32, tag="ht")
                    init = 0.0 if q % (NQ // B) == 0 else carry[:, j:j + 1]
                    nc.vector.tensor_tensor_scan(ht[:], abc[:], bb[:], init,
                                                 ALU.mult, ALU.add)
                    nc.vector.tensor_copy(carry[:, j:j + 1], ht[:, 511:512])
                    nc.sync.dma_start(h_T[j * 128:(j + 1) * 128, qsl], ht[:])
                    # --- hC + sum over n ---
                    hc = pst.tile([128, 512], f32, tag="hc")
                    nc.vector.tensor_tensor(hc[:], ht[:], cct[:], ALU.mult)
                    nc.tensor.matmul(psh[:], smat_t[:, j * 64:(j + 1) * 64],
                                     hc[:], start=(j == 0), stop=(j == NJ - 1))
                # y_ssm chunk
                nc.vector.scalar_tensor_tensor(ydss[DSH:128, qsl], psh[:],
                                               dbcol_t[:], ydss[0:DSH, qsl],
                                               ALU.add, ALU.add)
            for dst in range(W):
                nc.sync.dma_start(cc2_in[dst],
                                  ydss[DSH:128, dst * SL:(dst + 1) * SL])

        # ================= A2A + KCM tail =================
        nc.gpsimd.collective_compute(
            "AllToAll", ALU.bypass,
            ins=[cc2_in[:]], outs=[cc2_out[:]],
            replica_groups=RG,
        )
        with tc.tile_pool(name="kcm", bufs=1) as pk, \
             tc.tile_pool(name="kcm2", bufs=2) as pk2, \
             tc.tile_pool(name="psK", bufs=2, space="PSUM") as ppsK:
            ys = [pk.tile([128, SL], f32, name=f"ys{i}", tag=f"ys{i}") for i in range(4)]
            for i in range(4):
                nc.sync.dma_start(ys[i][0:DSH, :], cc2_out[2 * i])
                nc.sync.dma_start(ys[i][DSH:128, :], cc2_out[2 * i + 1])
            psk = ppsK.tile([DK + 1, SL], f32, tag="psk")
            for k in range(4):
                nc.tensor.matmul(psk[:], kg_t[k][:], ys[k][:],
                                 start=(k == 0), stop=(k == 3))
            kT = pk.tile([DK + 1, SL], f32, tag="kT")
            nc.scalar.activation(kT[:], psk[:], AF.Identity, bias=kbcol_t[:])
            psv = ppsK.tile([DV, SL], f32, tag="psv")
            nc.tensor.matmul(psv[:], mw_t[:], kT[0:DK, :], start=True, stop=True)
            vT = pk.tile([DV, SL], f32, tag="vT")
            nc.scalar.activation(vT[:], psv[:], AF.Copy)
            gr = pk.tile([1, SL], f32, tag="gr")
            nc.scalar.activation(gr[:], kT[DK:DK + 1, :], AF.Sigmoid)
            gb = pk.tile([128, SL], f32, tag="gb")
            nc.gpsimd.partition_broadcast(gb[:], gr[:])
            for m in range(4):
                psm = ppsK.tile([128, SL], f32, tag="psm")
                nc.tensor.matmul(psm[:], rcw_t[m][:], vT[:], start=True,
                                 stop=True)
                hm = pk2.tile([128, SL], f32, tag="hm")
                nc.scalar.activation(hm[:], psm[:], AF.Identity,
                                     bias=rbcol_t[:, m:m + 1])
                t2 = pk2.tile([128, SL], f32, tag="t2")
                nc.vector.tensor_tensor(t2[:], hm[:], gb[:], ALU.mult)
                yb = pk2.tile([128, SL], f32, tag="yb")
                nc.vector.tensor_tensor(yb[:], t2[:], ys[m][:], ALU.add)
                nc.sync.dma_start(y_T[m * 128:(m + 1) * 128, :], yb[:])

    nc.finalize()
    return nc


def kernel(**inputs):
    f32 = np.float32
    x_seq = np.asarray(inputs['x_seq'], f32)
    A_log = np.asarray(inputs['A_log'], f32)
    gamma = np.asarray(inputs['ln_gamma'], f32)
    beta = np.asarray(inputs['ln_beta'], f32)
    delta_W = np.asarray(inputs['delta_W'], f32)
    delta_b = np.asarray(inputs['delta_b'], f32)
    B_W = np.asarray(inputs['B_W'], f32)
    B_b = np.asarray(inputs['B_b'], f32)
    C_W = np.asarray(inputs['C_W'], f32)
    C_b = np.asarray(inputs['C_b'], f32)
    D_W = np.asarray(inputs['D_W'], f32)
    D_b = np.asarray(inputs['D_b'], f32)
    D_scale = np.asarray(inputs['D_scale'], f32)
    key_W = np.asarray(inputs['key_W'], f32)
    key_b = np.asarray(inputs['key_b'], f32)
    recall_W = np.asarray(inputs['recall_W'], f32)
    recall_b = np.asarray(inputs['recall_b'], f32)
    gate_W = np.asarray(inputs['gate_W'], f32)
    gate_b = np.asarray(inputs['gate_b'], f32)
    M = np.asarray(inputs['M'], f32)

    # rank-1 decay requires constant A_log (true for this problem's inputs)
    assert float(np.ptp(A_log)) == 0.0, "kernel specialized for constant A_log"
    a0 = float(-np.exp(A_log.reshape(-1)[0]))
    db_f = float(delta_b[0] + beta @ delta_W[:, 0])

    # folded weights
    BW_g = (B_W * gamma[:, None]).astype(f32)
    BB_f = (B_b + beta @ B_W).astype(f32)
    CW_g = (C_W * gamma[:, None]).astype(f32)
    CB_f = (C_b + beta @ C_W).astype(f32)
    dW_g = (delta_W * gamma[:, None]).astype(f32)
    DW_s = (D_W * D_scale[None, :]).astype(f32)
    Dcol = DW_s.sum(0).astype(f32)
    Db_s = (D_b * D_scale).astype(f32)
    keyg_W = np.concatenate([key_W, gate_W], 1).astype(f32)
    kb = np.concatenate([key_b, gate_b]).astype(f32)
    smat = np.zeros((128, 512), f32)
    for j in range(NJ):
        for p in range(128):
            smat[p, j * 64 + 8 * j + p // 16] = 1.0
    ident = np.eye(128, dtype=f32)
    x_flat = np.ascontiguousarray(x_seq.reshape(BT, DX))

    key = (a0, db_f)
    if key not in _CACHE:
        _CACHE[key] = _build(a0, db_f)
    nc = _CACHE[key]

    in_maps = []
    for c in range(W):
        dn0 = c * DNSH
        d0 = c * DSH
        im = {
            "x_sl": np.ascontiguousarray(x_flat[c * SL:(c + 1) * SL]),
            "bw": np.ascontiguousarray(BW_g[:, dn0:dn0 + DNSH]),
            "cw": np.ascontiguousarray(CW_g[:, dn0:dn0 + DNSH]),
            "dwp": np.ascontiguousarray(DW_s[:, d0:d0 + DSH]),
            "dwd": dW_g,
            "bbf": np.ascontiguousarray(
                BB_f[dn0:dn0 + DNSH].reshape(NJ, 128).T),
            "cbf": np.ascontiguousarray(
                CB_f[dn0:dn0 + DNSH].reshape(NJ, 128).T),
            "dbcol": np.ascontiguousarray(Db_s[d0:d0 + DSH, None]),
            "dccol": np.ascontiguousarray(Dcol[d0:d0 + DSH, None]),
            "kgw": keyg_W,
            "kbcol": np.ascontiguousarray(kb[:, None]),
            "mw": M,
            "rcw": recall_W,
            "rbcol": np.ascontiguousarray(recall_b.reshape(4, 128).T),
            "smat": smat,
            "identf": ident,
        }
        in_maps.append(im)

    from concourse.bass_utils import run_bass_kernel_spmd
    import os
    kw = {}
    if os.environ.get("KERNEL_TMPDIR"):
        kw["tmpdir"] = os.environ["KERNEL_TMPDIR"]
    res = run_bass_kernel_spmd(nc, in_maps, core_ids=list(range(W)), **kw)
    global LAST_RESULT
    LAST_RESULT = res
    results = res.results

    hT = np.concatenate([np.asarray(results[c]["h_T"]) for c in range(W)], 0)
    h_seq = np.ascontiguousarray(
        hT.reshape(DH, N, B, T).transpose(2, 3, 0, 1)).astype(f32)
    y_flat = np.concatenate(
        [np.asarray(results[c]["y_T"]).T for c in range(W)], 0)
    y = np.ascontiguousarray(y_flat.reshape(B, T, DH)).astype(f32)
    return y, h_seq
